# revision 15
# baseline (speedup 1.0000x reference)
"""Sparse attention (template/search) Trainium2 Bass kernel.

Problem: B=128, N=320 (T=64 template + S=256 search), C=768, H=12, d=64.
  x = concat(x1[:, :64], x2[:, 64:])
  qkv = x @ qkv_w.T ; per-head attention (template->template, search->all)
  out = attn @ proj_w.T + proj_b
Pure data parallel over batch: 16 batches per core on 8 cores.

Dataflow (per batch, all feature-major / "transposed" on chip, bf16
matmul operands, f32 PSUM accumulation):
  xT [C, N] --wqk--> qkT [2C rows, N]      (12 m-tiles, free dim 320)
  xT --wv--> v token-major per-head 65-wide blocks ([64 v cols | ones])
  scores^T [s, t] per head pair, quadrant-packed (two heads at
  tile_position row offsets 0/64 writing different PSUM banks so they
  stream concurrently); template scores ride the spare columns of the
  third s-tile.  Exp (ACT) -> bf16 tiles; attn @ [v|1] puts the softmax
  numerator in rows 0..63 and the denominator in row 64 of a psA tile.
  Normalization: DVE copies the denominator row to SBUF partition 0
  (the custom reciprocal misreads PSUM and non-zero partition offsets),
  DVE reciprocal_approx_fast (NOT the 8-cycle/elem iterative divide),
  GpSimd partition-broadcast, DVE multiply -> attn bf16.  The proj bias
  is fused into the DVE PSUM->SBUF copy as a tensor_scalar_add.

Scheduling: the whole kernel is software-pipelined at depth 2 in
uniform slots -- window b emits, per slot p in 0..5:
  [qkv m-tiles 2p,2p+1 of batch b] [scores pair p of batch b-1]
  [attnV+norm pair p-1 of b-1] [proj m-tile p-1 of batch b-2]
with batch b's v-tiles and the attention/proj stragglers in the window
tail.  This keeps the PE's activity window saturated with long streams
(the HAM clock gate stays at 8/8 = 2.4GHz) and spreads the attention
phase's ACT/DVE/GpSimd work (Exp, denominator copies, reciprocals,
normalization multiplies) over the full window instead of cramming it
into a short attention phase where it stalls the PE.  Engine budget per
window (~40us): ACT = Exps + v copies ~18us, DVE = qk copies + den +
recip + mul + yt ~27us, GpSimd = broadcasts ~9us, all under the PE.

PSUM discipline: two matmuls that can execute concurrently on the PE
(disjoint row groups) must never target the same PSUM bank.  The only
row-disjoint concurrent writers are the quadrant-packed score matmuls,
which write different banks by construction; every other matmul spans
row 0+ and is row-serialized with its neighbors.  psA (1-bank [128,512]
tiles, bufs=4) rotates qkv/v/proj accumulators and attnV outputs; psB
(2-bank [128,1024], bufs=2) holds scores; all 8 banks in use.
"""

import numpy as np
import ml_dtypes

import concourse.bass as bass
import concourse.bacc as bacc
import concourse.mybir as mybir
from concourse.tile import TileContext
from concourse.bass_utils import run_bass_kernel_spmd

f32 = mybir.dt.float32
bf16 = mybir.dt.bfloat16
i32 = mybir.dt.int32
Exp = mybir.ActivationFunctionType.Exp


B, N, C = 128, 320, 768
H, D = 12, 64
T, S = 64, 256
N_CORES = 8
BPC = B // N_CORES  # batches per core

NCT = C // 128            # 6 c-tiles of 128
NQK = 2 * C // 128        # 12 qk row-tiles
NPAIR = H // 2            # 6 head pairs
S_TILES = [(0, 128), (128, 128), (256, 64)]   # (s0, ssz) key-token tiles
SCALE = D ** -0.5
VW = 65                   # per-head V block width (64 v cols + ones)
V_GROUPS = [(0, 128, 0, 512), (0, 128, 512, 256),
            (1, 128, 0, 512), (1, 128, 512, 256),
            (2, 64, 0, 512), (2, 64, 512, 256)]  # (tt, tsz, c0, csz)
NP_BF16 = ml_dtypes.bfloat16


def build_bass(bpc: int = BPC, n_cores: int = N_CORES, reps: int = 1):
    nc = bacc.Bacc("TRN2", target_bir_lowering=False, debug=False,
                   num_devices=n_cores)

    # host-repacked so every DMA is contiguous per partition:
    #   xt[b, p, ct, t]      = x_featmaj[b, ct*128+p, t]
    #   wqk[p, j, ct, mm]    = qkv_w[j*256+mm, ct*128+p]   (m-chunk-major)
    #   wv[p, ct, m]         = qkv_w[2C+m, ct*128+p]
    #   wp[p, ct, m]         = proj_w[m, ct*128+p]
    xt_d = nc.declare_dram_parameter("xt", [bpc, 128, NCT, N], bf16,
                                     isOutput=False)
    wqk_d = nc.declare_dram_parameter("wqk", [128, 6, NCT, 256], bf16,
                                      isOutput=False)
    wv_d = nc.declare_dram_parameter("wv", [128, NCT, C], bf16,
                                     isOutput=False)
    wp_d = nc.declare_dram_parameter("wp", [128, NCT, C], bf16,
                                     isOutput=False)
    # pbt[p, m] = proj_b[m*128 + p]
    pb_d = nc.declare_dram_parameter("pbt", [128, NCT], f32, isOutput=False)
    r_d = None
    if reps == 0:   # timing harness: runtime iteration count
        r_d = nc.declare_dram_parameter("reps_in", [1, 1], i32, isOutput=False)
    y_d = nc.declare_dram_parameter("y", [bpc, C, N], f32, isOutput=True)

    with TileContext(nc) as tc:
        with (
            tc.tile_pool(name="wpool", bufs=1) as wpool,
            tc.tile_pool(name="xpool", bufs=3) as xpool,
            tc.tile_pool(name="qkpool", bufs=2) as qkpool,
            tc.tile_pool(name="vpool", bufs=2) as vpool,
            tc.tile_pool(name="epool", bufs=6) as epool,
            tc.tile_pool(name="apool", bufs=3) as apool,
            tc.tile_pool(name="rpool", bufs=8) as rpool,
            tc.tile_pool(name="bpool", bufs=6) as bpool,
            tc.tile_pool(name="ypool", bufs=3) as ypool,
            tc.tile_pool(name="psA", bufs=4, space="PSUM") as psA,
            tc.tile_pool(name="psB", bufs=2, space="PSUM") as psB,
        ):
            # ---- persistent weights ----
            # wqk split into 6 m-chunks so the first qkv m-tiles can start
            # as soon as chunk 0 lands (the xt[0] DMA is issued first, in
            # body()); wv/wp/pb follow and arrive well before first use.
            wqk_sb = wpool.tile([128, 6, NCT, 256], bf16)  # lhsT for q,k
            wv_sb = wpool.tile([128, NCT, C], bf16)        # rhs for v
            wp_sb = wpool.tile([128, NCT, C], bf16)        # lhsT for proj
            pb_sb = wpool.tile([128, NCT], f32)

            def weight_dmas():
                for j in range(6):
                    nc.sync.dma_start(out=wqk_sb[:, j], in_=wqk_d[:, j])
                nc.sync.dma_start(out=wv_sb[:], in_=wv_d[:])
                nc.sync.dma_start(out=wp_sb[:], in_=wp_d[:])
                nc.sync.dma_start(out=pb_sb[:], in_=pb_d[:])
            rv = None
            if reps == 0:
                r_sb = wpool.tile([1, 1], i32)
                nc.sync.dma_start(out=r_sb[:], in_=r_d[:])
                tmp = nc.alloc_registers("reps_regs")
                nc.regs_load(tmp, r_sb[0:1, 0:1])
                rv = nc.snap(tmp, donate=True, min_val=1, max_val=4096)

            def emit_qkv_m(xt_sb, qk_sb, m):
                pacc = psA.tile([128, 512], f32, tag="pacc")
                for ct in range(NCT):
                    nc.tensor.matmul(
                        pacc[:, 0:N],
                        wqk_sb[:, m // 2, ct,
                               (m % 2) * 128:(m % 2) * 128 + 128],
                        xt_sb[:, ct, :],
                        start=(ct == 0), stop=(ct == NCT - 1))
                nc.vector.tensor_copy(qk_sb[:, m, :], pacc[:, 0:N])

            def emit_v_g(xt_sb, v_sb, g):
                tt, tsz, c0, csz = V_GROUPS[g]
                pacc = psA.tile([128, 512], f32, tag="pacc")
                for ct in range(NCT):
                    nc.tensor.matmul(
                        pacc[0:tsz, 0:csz],
                        xt_sb[:, ct, tt * 128:tt * 128 + tsz],
                        wv_sb[:, ct, c0:c0 + csz],
                        start=(ct == 0), stop=(ct == NCT - 1))
                nh = csz // D
                h0 = c0 // D
                dst = v_sb[0:tsz, tt, h0 * VW:(h0 + nh) * VW] \
                    .rearrange("p (h c) -> p h c", c=VW)[:, :, 0:D]
                src = pacc[0:tsz, 0:csz].rearrange("p (h c) -> p h c", c=D)
                nc.scalar.copy(dst, src)

            def emit_v_ones(v_sb, first_gen):
                # ones for tt 0,1 (all 128 rows) and tt 2 rows 0:64 only:
                # rows 64:128 of tt 2 stay ZERO so the st2 attnV matmul can
                # run full-row (K=128) — its 64 pad rows contribute nothing.
                ones01 = v_sb[:, 0:2, :].rearrange(
                    "p t (h c) -> p t h c", c=VW)[:, :, :, D:VW]
                nc.vector.memset(ones01, 1.0)
                ones2 = v_sb[0:64, 2:3, :].rearrange(
                    "p t (h c) -> p t h c", c=VW)[:, :, :, D:VW]
                nc.vector.memset(ones2, 1.0)
                if first_gen:   # zero the pad rows once per vpool buffer
                    nc.vector.memset(v_sb[64:128, 2, :], 0.0)

            def emit_sc(qk_sb, p):
                """Score matmuls + Exp for pair p. Returns exp tiles.

                The first s-tile streams ALL 320 queries: its columns
                0:64 carry the template queries' scores against keys
                0:128, of which keys 64:128 are masked by zeroing the
                exp tile (one cheap DVE memset); attnV's first matmul
                then accumulates the template numerators/denominators
                for free, so no separate template matmuls exist."""
                mq, mk = p, NPAIR + p
                exps = []
                for st, (s0, ssz) in enumerate(S_TILES):
                    first = (st == 0)
                    w = 320 if first else 256
                    q0 = 0 if first else T
                    sc = psB.tile([128, 1024], f32, tag="sc")
                    for i in (0, 1):
                        pof = 64 * i
                        nc.tensor.matmul(
                            sc[0:ssz, 512 * i:512 * i + w],
                            qk_sb[pof:pof + 64, mk, s0:s0 + ssz],
                            qk_sb[pof:pof + 64, mq, q0:N],
                            start=True, stop=True,
                            tile_position=(pof, 0))
                    ex = epool.tile([128, 2, 320], bf16, tag="ex")
                    gap_in = bass.AP(
                        tensor=sc.tensor, offset=sc.offset,
                        ap=[sc.ap[0], [512, 2], [1, w]])
                    nc.scalar.activation(out=ex[0:ssz, :, 0:w],
                                         in_=gap_in[0:ssz],
                                         func=Exp, scale=SCALE)
                    if first:
                        nc.vector.memset(ex[64:128, :, 0:T], 0.0)
                    exps.append(ex)
                return exps

            def emit_av(v_sb, attn_sb, p, exps):
                """attn @ [v | 1] + normalization for pair p.

                All three matmuls run full-row (K=128): the st2 tile's v
                rows 64:128 are zero-padded and the matching exp rows are
                zero/stale-finite, so the pad contributes nothing.  Full-row
                LDWEIGHTS are background-buffer eligible and pipeline behind
                the in-flight matmul instead of stalling on its drain."""
                for i, h in enumerate((2 * p, 2 * p + 1)):
                    O = psA.tile([128, 512], f32, tag="pacc")
                    for st, (s0, ssz) in enumerate(S_TILES):
                        first = (st == 0)
                        nc.tensor.matmul(
                            O[0:VW, 0:N] if first else O[0:VW, T:N],
                            v_sb[0:128, st, h * VW:(h + 1) * VW],
                            exps[st][0:128, i, 0:320 if first else 256],
                            start=first,
                            stop=(st == len(S_TILES) - 1))

                    den = rpool.tile([1, N], f32, tag="den")
                    nc.scalar.copy(den[0:1, :], O[64:65, 0:N])
                    rec = rpool.tile([1, N], f32, tag="rec")
                    nc.vector.reciprocal_approx_fast(out=rec[0:1, :],
                                                     in_=den[0:1, :])
                    rb = bpool.tile([64, N], f32, tag="rb")
                    nc.gpsimd.partition_broadcast(rb[0:64, :], rec[0:1, :])
                    nc.vector.tensor_mul(
                        attn_sb[64 * i:64 * i + 64, p, :],
                        O[0:64, 0:N], rb[0:64, :])

            def emit_pj(attn_sb, b, m):
                yp = psA.tile([128, 512], f32, tag="pacc")
                for ct in range(NCT):
                    nc.tensor.matmul(
                        yp[:, 0:N],
                        wp_sb[:, ct, m * 128:(m + 1) * 128],
                        attn_sb[:, ct, :],
                        start=(ct == 0), stop=(ct == NCT - 1))
                yt_sb = ypool.tile([128, N], f32, tag="yt")
                nc.vector.tensor_scalar_add(yt_sb[:], yp[:, 0:N],
                                            pb_sb[:, m:m + 1])
                nc.sync.dma_start(out=y_d[b, m * 128:(m + 1) * 128, :],
                                  in_=yt_sb[:])

            def body(_iv=None):
                xts = {}

                def get_xt(b):
                    if b not in xts and 0 <= b < bpc:
                        t = xpool.tile([128, NCT, N], bf16, name="xt_sb")
                        nc.sync.dma_start(out=t[:], in_=xt_d[b])
                        xts[b] = t
                    return xts.get(b)

                get_xt(0)
                weight_dmas()
                # one-time: zero rows 64:128 of all 6 rotating exp buffers so
                # the K=128-padded st2 attnV matmul (which streams those rows
                # against zeroed v rows) can never hit NaN-bit garbage there.
                for _ in range(6):
                    ex_init = epool.tile([128, 2, 320], bf16, tag="ex",
                                         name="ex_init")
                    nc.vector.memset(ex_init[64:128, :, :], 0.0)
                projq = []      # [(attn_sb, b)] awaiting projection

                for b in range(bpc):
                    # 1-window-lag pipeline: window b computes qkv(b),
                    # v(b), and batch b's OWN attention (scores one slot
                    # behind the pair's qkv m-tiles -- emitted pair-major
                    # as (p, 6+p) -- attnV three slots behind), with the
                    # projection of batch b-1 interleaved.  The drain is
                    # just the final batch's projection.
                    xt_sb = get_xt(b)
                    get_xt(b + 1)   # prefetch next batch's input early
                    qk_sb = qkpool.tile([128, NQK, N], bf16)
                    v_sb = vpool.tile([128, 3, H * VW], bf16)
                    attn_sb = apool.tile([128, NPAIR, N], bf16)
                    pj = projq.pop(0) if len(projq) > 0 else None
                    # v first: attnV pair 0 (slot 4) needs every v group
                    for g in range(6):
                        emit_v_g(xt_sb, v_sb, g)
                    emit_v_ones(v_sb, b < 2)
                    exps = {}
                    for p in range(NPAIR):
                        emit_qkv_m(xt_sb, qk_sb, p)
                        emit_qkv_m(xt_sb, qk_sb, NPAIR + p)
                        if p >= 1:
                            exps[p - 1] = emit_sc(qk_sb, p - 1)
                        if p >= 3:
                            emit_av(v_sb, attn_sb, p - 3, exps.pop(p - 3))
                        if pj is not None and p >= 1:
                            emit_pj(*pj, m=p - 1)
                    exps[NPAIR - 1] = emit_sc(qk_sb, NPAIR - 1)
                    emit_av(v_sb, attn_sb, NPAIR - 3, exps.pop(NPAIR - 3))
                    emit_av(v_sb, attn_sb, NPAIR - 2, exps.pop(NPAIR - 2))
                    if pj is not None:
                        emit_pj(*pj, m=NPAIR - 1)
                    emit_av(v_sb, attn_sb, NPAIR - 1, exps.pop(NPAIR - 1))
                    projq.append((attn_sb, b))
                # drain: only the final batch's projection remains.
                for m in range(NCT):
                    emit_pj(*projq[0], m=m)

            if reps == 1:
                body()
            elif reps == 0:
                with tc.For_i(0, rv, 1) as _i:
                    body(_i)
            else:
                with tc.For_i(0, reps, 1) as _i:
                    body(_i)

    nc.compile()
    return nc


_NC_CACHE = {}


def _get_nc(bpc: int = BPC):
    if bpc not in _NC_CACHE:
        _NC_CACHE[bpc] = build_bass(bpc)
    return _NC_CACHE[bpc]


def make_in_maps(x1, x2, qkv_w, proj_w, proj_b, n_cores=N_CORES):
    x1 = np.asarray(x1, dtype=np.float32)
    x2 = np.asarray(x2, dtype=np.float32)
    qkv_w = np.asarray(qkv_w, dtype=np.float32)
    proj_w = np.asarray(proj_w, dtype=np.float32)
    proj_b = np.asarray(proj_b, dtype=np.float32)

    b = x1.shape[0]
    xt = np.empty((b, C, N), dtype=NP_BF16)
    xt[:, :, :T] = x1[:, :T, :].transpose(0, 2, 1).astype(NP_BF16)
    xt[:, :, T:] = x2[:, T:, :].transpose(0, 2, 1).astype(NP_BF16)
    # [b, 128, NCT, N]: contiguous per partition for the DMA
    xt = np.ascontiguousarray(
        xt.reshape(b, NCT, 128, N).transpose(0, 2, 1, 3))

    # wqk[p, j, ct, mm] = qkv_w[j*256+mm, ct*128+p]
    wqk = np.ascontiguousarray(
        qkv_w[:2 * C].reshape(6, 256, NCT, 128).transpose(3, 0, 2, 1)
    ).astype(NP_BF16)
    # wv[p, ct, m] = qkv_w[2C+m, ct*128+p]
    wv = np.ascontiguousarray(
        qkv_w[2 * C:].reshape(C, NCT, 128).transpose(2, 1, 0)).astype(NP_BF16)
    # wp[p, ct, m] = proj_w[m, ct*128+p]
    wp = np.ascontiguousarray(
        proj_w.reshape(C, NCT, 128).transpose(2, 1, 0)).astype(NP_BF16)
    pbt = np.ascontiguousarray(proj_b.reshape(NCT, 128).T)  # [128, NCT] f32

    bpc = b // n_cores
    return [
        {"xt": xt[c * bpc:(c + 1) * bpc], "wqk": wqk, "wv": wv, "wp": wp,
         "pbt": pbt}
        for c in range(n_cores)
    ], bpc


def kernel(x1, x2, qkv_w, proj_w, proj_b):
    in_maps, bpc = make_in_maps(x1, x2, qkv_w, proj_w, proj_b)
    nc = _get_nc(bpc)
    res = run_bass_kernel_spmd(nc, in_maps, list(range(N_CORES)))
    yt = np.concatenate([res.results[c]["y"] for c in range(N_CORES)], axis=0)
    return np.ascontiguousarray(yt.transpose(0, 2, 1))



# revision 17
# speedup vs baseline: 1.1616x; 1.1616x over previous
"""Sparse attention (template/search) Trainium2 Bass kernel.

Problem: B=128, N=320 (T=64 template + S=256 search), C=768, H=12, d=64.
  x = concat(x1[:, :64], x2[:, 64:])
  qkv = x @ qkv_w.T ; per-head attention (template->template, search->all)
  out = attn @ proj_w.T + proj_b
Pure data parallel over batch: 16 batches per core on 8 cores.

Dataflow (per batch, all feature-major / "transposed" on chip, bf16
matmul operands, f32 PSUM accumulation):
  xT [C, N] --wqk--> qkT [2C rows, N]      (12 m-tiles, free dim 320)
  xT --wv--> v token-major per-head 65-wide blocks ([64 v cols | ones])
  scores^T [s, t] per head pair, quadrant-packed (two heads at
  tile_position row offsets 0/64 writing different PSUM banks so they
  stream concurrently); template scores ride the spare columns of the
  third s-tile.  Exp (ACT) -> bf16 tiles; attn @ [v|1] puts the softmax
  numerator in rows 0..63 and the denominator in row 64 of a psA tile.
  ALL attnV matmuls run full-row (K=128): the third s-tile's v rows
  64:128 are zero-padded (and the matching exp-buffer rows one-time
  zeroed at kernel start) because partial-row LDWEIGHTS cannot use the
  PE's background weight buffer and stall on matmul drains -- padding
  costs nothing (matmul time is free-dim-only) and removed ~85us.
  Normalization: ACT copies the denominator row to SBUF partition 0
  (the custom reciprocal misreads PSUM and non-zero partition offsets),
  DVE reciprocal_approx_fast (NOT the 8-cycle/elem iterative divide),
  GpSimd partition-broadcast, DVE multiply -> attn bf16.  The proj bias
  is fused into the DVE PSUM->SBUF copy as a tensor_scalar_add.
  All DRAM parameters are host-repacked so every DMA line is contiguous
  per partition (3-4KB lines); wqk arrives in 6 m-chunks after xt[0] so
  the first matmul starts ~3us in.

Scheduling: the whole kernel is software-pipelined at depth 2 in
uniform slots -- window b emits, per slot p in 0..5:
  [qkv m-tiles 2p,2p+1 of batch b] [scores pair p of batch b-1]
  [attnV+norm pair p-1 of b-1] [proj m-tile p-1 of batch b-2]
with batch b's v-tiles and the attention/proj stragglers in the window
tail.  This keeps the PE's activity window saturated with long streams
(the HAM clock gate stays at 8/8 = 2.4GHz) and spreads the attention
phase's ACT/DVE/GpSimd work (Exp, denominator copies, reciprocals,
normalization multiplies) over the full window instead of cramming it
into a short attention phase where it stalls the PE.  Engine budget per
window (~40us): ACT = Exps + v copies ~18us, DVE = qk copies + den +
recip + mul + yt ~27us, GpSimd = broadcasts ~9us, all under the PE.

PSUM discipline: two matmuls that can execute concurrently on the PE
(disjoint row groups) must never target the same PSUM bank.  The only
row-disjoint concurrent writers are the quadrant-packed score matmuls,
which write different banks by construction; every other matmul spans
row 0+ and is row-serialized with its neighbors.  psA (1-bank [128,512]
tiles, bufs=4) rotates qkv/v/proj accumulators and attnV outputs; psB
(2-bank [128,1024], bufs=2) holds scores; all 8 banks in use.
"""

import numpy as np
import ml_dtypes

import concourse.bass as bass
import concourse.bacc as bacc
import concourse.mybir as mybir
from concourse.tile import TileContext
from concourse.bass_utils import run_bass_kernel_spmd

f32 = mybir.dt.float32
bf16 = mybir.dt.bfloat16
i32 = mybir.dt.int32
Exp = mybir.ActivationFunctionType.Exp


B, N, C = 128, 320, 768
H, D = 12, 64
T, S = 64, 256
N_CORES = 8
BPC = B // N_CORES  # batches per core

NCT = C // 128            # 6 c-tiles of 128
NQK = 2 * C // 128        # 12 qk row-tiles
NPAIR = H // 2            # 6 head pairs
S_TILES = [(0, 128), (128, 128), (256, 64)]   # (s0, ssz) key-token tiles
SCALE = D ** -0.5
VW = 65                   # per-head V block width (64 v cols + ones)
V_GROUPS = [(0, 128, 0, 512), (0, 128, 512, 256),
            (1, 128, 0, 512), (1, 128, 512, 256),
            (2, 64, 0, 512), (2, 64, 512, 256)]  # (tt, tsz, c0, csz)
NP_BF16 = ml_dtypes.bfloat16


def build_bass(bpc: int = BPC, n_cores: int = N_CORES, reps: int = 1):
    nc = bacc.Bacc("TRN2", target_bir_lowering=False, debug=False,
                   num_devices=n_cores)

    # host-repacked so every DMA is contiguous per partition:
    #   xt[b, p, ct, t]      = x_featmaj[b, ct*128+p, t]
    #   wqk[p, j, ct, mm]    = qkv_w[j*256+mm, ct*128+p]   (m-chunk-major)
    #   wv[p, ct, m]         = qkv_w[2C+m, ct*128+p]
    #   wp[p, ct, m]         = proj_w[m, ct*128+p]
    xt_d = nc.declare_dram_parameter("xt", [bpc, 128, NCT, N], bf16,
                                     isOutput=False)
    wqk_d = nc.declare_dram_parameter("wqk", [128, 6, NCT, 256], bf16,
                                      isOutput=False)
    wv_d = nc.declare_dram_parameter("wv", [128, NCT, C], bf16,
                                     isOutput=False)
    wp_d = nc.declare_dram_parameter("wp", [128, NCT, C], bf16,
                                     isOutput=False)
    # pbt[p, m] = proj_b[m*128 + p]
    pb_d = nc.declare_dram_parameter("pbt", [128, NCT], f32, isOutput=False)
    r_d = None
    if reps == 0:   # timing harness: runtime iteration count
        r_d = nc.declare_dram_parameter("reps_in", [1, 1], i32, isOutput=False)
    y_d = nc.declare_dram_parameter("y", [bpc, C, N], f32, isOutput=True)

    with TileContext(nc) as tc:
        with (
            tc.tile_pool(name="wpool", bufs=1) as wpool,
            tc.tile_pool(name="xpool", bufs=3) as xpool,
            tc.tile_pool(name="qkpool", bufs=2) as qkpool,
            tc.tile_pool(name="vpool", bufs=2) as vpool,
            tc.tile_pool(name="epool", bufs=6) as epool,
            tc.tile_pool(name="apool", bufs=3) as apool,
            tc.tile_pool(name="rpool", bufs=8) as rpool,
            tc.tile_pool(name="bpool", bufs=6) as bpool,
            tc.tile_pool(name="ypool", bufs=3) as ypool,
            tc.tile_pool(name="psA", bufs=4, space="PSUM") as psA,
            tc.tile_pool(name="psB", bufs=2, space="PSUM") as psB,
        ):
            # ---- persistent weights ----
            # wqk split into 6 m-chunks so the first qkv m-tiles can start
            # as soon as chunk 0 lands (the xt[0] DMA is issued first, in
            # body()); wv/wp/pb follow and arrive well before first use.
            wqk_sb = wpool.tile([128, 6, NCT, 256], bf16)  # lhsT for q,k
            wv_sb = wpool.tile([128, NCT, C], bf16)        # rhs for v
            wp_sb = wpool.tile([128, NCT, C], bf16)        # lhsT for proj
            pb_sb = wpool.tile([128, NCT], f32)

            def weight_dmas():
                for j in range(6):
                    nc.sync.dma_start(out=wqk_sb[:, j], in_=wqk_d[:, j])
                nc.sync.dma_start(out=wv_sb[:], in_=wv_d[:])
                nc.sync.dma_start(out=wp_sb[:], in_=wp_d[:])
                nc.sync.dma_start(out=pb_sb[:], in_=pb_d[:])
            rv = None
            if reps == 0:
                r_sb = wpool.tile([1, 1], i32)
                nc.sync.dma_start(out=r_sb[:], in_=r_d[:])
                tmp = nc.alloc_registers("reps_regs")
                nc.regs_load(tmp, r_sb[0:1, 0:1])
                rv = nc.snap(tmp, donate=True, min_val=1, max_val=4096)

            def emit_qkv_m(xt_sb, qk_sb, m):
                pacc = psA.tile([128, 512], f32, tag="pacc")
                for ct in range(NCT):
                    nc.tensor.matmul(
                        pacc[:, 0:N],
                        wqk_sb[:, m // 2, ct,
                               (m % 2) * 128:(m % 2) * 128 + 128],
                        xt_sb[:, ct, :],
                        start=(ct == 0), stop=(ct == NCT - 1))
                nc.vector.tensor_copy(qk_sb[:, m, :], pacc[:, 0:N])

            def emit_v_g(xt_sb, v_sb, g):
                tt, tsz, c0, csz = V_GROUPS[g]
                pacc = psA.tile([128, 512], f32, tag="pacc")
                for ct in range(NCT):
                    nc.tensor.matmul(
                        pacc[0:tsz, 0:csz],
                        xt_sb[:, ct, tt * 128:tt * 128 + tsz],
                        wv_sb[:, ct, c0:c0 + csz],
                        start=(ct == 0), stop=(ct == NCT - 1))
                nh = csz // D
                h0 = c0 // D
                dst = v_sb[0:tsz, tt, h0 * VW:(h0 + nh) * VW] \
                    .rearrange("p (h c) -> p h c", c=VW)[:, :, 0:D]
                src = pacc[0:tsz, 0:csz].rearrange("p (h c) -> p h c", c=D)
                nc.scalar.copy(dst, src)

            def emit_v_ones(v_sb, first_gen):
                # ones for tt 0,1 (all 128 rows) and tt 2 rows 0:64 only:
                # rows 64:128 of tt 2 stay ZERO so the st2 attnV matmul can
                # run full-row (K=128) — its 64 pad rows contribute nothing.
                ones01 = v_sb[:, 0:2, :].rearrange(
                    "p t (h c) -> p t h c", c=VW)[:, :, :, D:VW]
                nc.vector.memset(ones01, 1.0)
                ones2 = v_sb[0:64, 2:3, :].rearrange(
                    "p t (h c) -> p t h c", c=VW)[:, :, :, D:VW]
                nc.vector.memset(ones2, 1.0)
                if first_gen:   # zero the pad rows once per vpool buffer
                    nc.vector.memset(v_sb[64:128, 2, :], 0.0)

            def emit_sc(qk_sb, p):
                """Score matmuls + Exp for pair p. Returns exp tiles.

                The first s-tile streams ALL 320 queries: its columns
                0:64 carry the template queries' scores against keys
                0:128, of which keys 64:128 are masked by zeroing the
                exp tile (one cheap DVE memset); attnV's first matmul
                then accumulates the template numerators/denominators
                for free, so no separate template matmuls exist."""
                mq, mk = p, NPAIR + p
                exps = []
                for st, (s0, ssz) in enumerate(S_TILES):
                    first = (st == 0)
                    w = 320 if first else 256
                    q0 = 0 if first else T
                    sc = psB.tile([128, 1024], f32, tag="sc")
                    for i in (0, 1):
                        pof = 64 * i
                        nc.tensor.matmul(
                            sc[0:ssz, 512 * i:512 * i + w],
                            qk_sb[pof:pof + 64, mk, s0:s0 + ssz],
                            qk_sb[pof:pof + 64, mq, q0:N],
                            start=True, stop=True,
                            tile_position=(pof, 0))
                    ex = epool.tile([128, 2, 320], bf16, tag="ex")
                    gap_in = bass.AP(
                        tensor=sc.tensor, offset=sc.offset,
                        ap=[sc.ap[0], [512, 2], [1, w]])
                    nc.scalar.activation(out=ex[0:ssz, :, 0:w],
                                         in_=gap_in[0:ssz],
                                         func=Exp, scale=SCALE)
                    if first:
                        nc.vector.memset(ex[64:128, :, 0:T], 0.0)
                    exps.append(ex)
                return exps

            def emit_av(v_sb, attn_sb, p, exps):
                """attn @ [v | 1] + normalization for pair p.

                All three matmuls run full-row (K=128): the st2 tile's v
                rows 64:128 are zero-padded and the matching exp rows are
                zero/stale-finite, so the pad contributes nothing.  Full-row
                LDWEIGHTS are background-buffer eligible and pipeline behind
                the in-flight matmul instead of stalling on its drain."""
                for i, h in enumerate((2 * p, 2 * p + 1)):
                    O = psA.tile([128, 512], f32, tag="pacc")
                    for st, (s0, ssz) in enumerate(S_TILES):
                        first = (st == 0)
                        nc.tensor.matmul(
                            O[0:VW, 0:N] if first else O[0:VW, T:N],
                            v_sb[0:128, st, h * VW:(h + 1) * VW],
                            exps[st][0:128, i, 0:320 if first else 256],
                            start=first,
                            stop=(st == len(S_TILES) - 1))

                    den = rpool.tile([1, N], f32, tag="den")
                    nc.scalar.copy(den[0:1, :], O[64:65, 0:N])
                    rec = rpool.tile([1, N], f32, tag="rec")
                    nc.vector.reciprocal_approx_fast(out=rec[0:1, :],
                                                     in_=den[0:1, :])
                    rb = bpool.tile([64, N], f32, tag="rb")
                    nc.gpsimd.partition_broadcast(rb[0:64, :], rec[0:1, :])
                    nc.vector.tensor_mul(
                        attn_sb[64 * i:64 * i + 64, p, :],
                        O[0:64, 0:N], rb[0:64, :])

            def emit_pj(attn_sb, b, m):
                yp = psA.tile([128, 512], f32, tag="pacc")
                for ct in range(NCT):
                    nc.tensor.matmul(
                        yp[:, 0:N],
                        wp_sb[:, ct, m * 128:(m + 1) * 128],
                        attn_sb[:, ct, :],
                        start=(ct == 0), stop=(ct == NCT - 1))
                yt_sb = ypool.tile([128, N], f32, tag="yt")
                nc.vector.tensor_scalar_add(yt_sb[:], yp[:, 0:N],
                                            pb_sb[:, m:m + 1])
                nc.sync.dma_start(out=y_d[b, m * 128:(m + 1) * 128, :],
                                  in_=yt_sb[:])

            def body(_iv=None):
                xts = {}

                def get_xt(b):
                    if b not in xts and 0 <= b < bpc:
                        t = xpool.tile([128, NCT, N], bf16, name="xt_sb")
                        nc.sync.dma_start(out=t[:], in_=xt_d[b])
                        xts[b] = t
                    return xts.get(b)

                get_xt(0)
                weight_dmas()
                # one-time: zero rows 64:128 of all 6 rotating exp buffers so
                # the K=128-padded st2 attnV matmul (which streams those rows
                # against zeroed v rows) can never hit NaN-bit garbage there.
                for _ in range(6):
                    ex_init = epool.tile([128, 2, 320], bf16, tag="ex",
                                         name="ex_init")
                    nc.vector.memset(ex_init[64:128, :, :], 0.0)
                pend = None     # (qk_sb, v_sb, b)
                projq = []      # [(attn_sb, b)] awaiting projection

                def attn_window(xt_sb, qk_sb, v_sb, vb=None):
                    """One pipelined window: qkv/v of the current batch
                    (None for the drain window) + attention of pend +
                    projection of projq[0]."""
                    pqk, pv, pb_ = pend
                    attn_sb = apool.tile([128, NPAIR, N], bf16)
                    pj = projq.pop(0) if len(projq) > 0 else None
                    exps = {}
                    for p in range(NPAIR):
                        if qk_sb is not None:
                            emit_qkv_m(xt_sb, qk_sb, 2 * p)
                            emit_qkv_m(xt_sb, qk_sb, 2 * p + 1)
                        exps[p] = emit_sc(pqk, p)
                        if p >= 1:
                            emit_av(pv, attn_sb, p - 1, exps.pop(p - 1))
                            if pj is not None:
                                emit_pj(*pj, m=p - 1)
                    if v_sb is not None:
                        emit_v_g(xt_sb, v_sb, 0)
                        emit_v_g(xt_sb, v_sb, 1)
                    emit_av(pv, attn_sb, NPAIR - 1, exps.pop(NPAIR - 1))
                    if v_sb is not None:
                        emit_v_g(xt_sb, v_sb, 2)
                        emit_v_g(xt_sb, v_sb, 3)
                    if pj is not None:
                        emit_pj(*pj, m=NPAIR - 1)
                    if v_sb is not None:
                        emit_v_g(xt_sb, v_sb, 4)
                        emit_v_g(xt_sb, v_sb, 5)
                        emit_v_ones(v_sb, vb == 1)
                    projq.append((attn_sb, pb_))

                for b in range(bpc):
                    xt_sb = get_xt(b)
                    get_xt(b + 1)   # prefetch next batch's input early
                    qk_sb = qkpool.tile([128, NQK, N], bf16)
                    v_sb = vpool.tile([128, 3, H * VW], bf16)
                    if pend is None:
                        for m in range(NQK):
                            emit_qkv_m(xt_sb, qk_sb, m)
                        for g in range(6):
                            emit_v_g(xt_sb, v_sb, g)
                        emit_v_ones(v_sb, True)
                    else:
                        attn_window(xt_sb, qk_sb, v_sb, vb=b)
                    pend = (qk_sb, v_sb, b)
                # drain: attention of the last batch, then the two
                # outstanding projections.
                attn_window(None, None, None)
                for m in range(NCT):
                    emit_pj(*projq[0], m=m)

            if reps == 1:
                body()
            elif reps == 0:
                with tc.For_i(0, rv, 1) as _i:
                    body(_i)
            else:
                with tc.For_i(0, reps, 1) as _i:
                    body(_i)

    nc.compile()
    return nc


_NC_CACHE = {}


def _get_nc(bpc: int = BPC):
    if bpc not in _NC_CACHE:
        _NC_CACHE[bpc] = build_bass(bpc)
    return _NC_CACHE[bpc]


def make_in_maps(x1, x2, qkv_w, proj_w, proj_b, n_cores=N_CORES):
    x1 = np.asarray(x1, dtype=np.float32)
    x2 = np.asarray(x2, dtype=np.float32)
    qkv_w = np.asarray(qkv_w, dtype=np.float32)
    proj_w = np.asarray(proj_w, dtype=np.float32)
    proj_b = np.asarray(proj_b, dtype=np.float32)

    b = x1.shape[0]
    xt = np.empty((b, C, N), dtype=NP_BF16)
    xt[:, :, :T] = x1[:, :T, :].transpose(0, 2, 1).astype(NP_BF16)
    xt[:, :, T:] = x2[:, T:, :].transpose(0, 2, 1).astype(NP_BF16)
    # [b, 128, NCT, N]: contiguous per partition for the DMA
    xt = np.ascontiguousarray(
        xt.reshape(b, NCT, 128, N).transpose(0, 2, 1, 3))

    # wqk[p, j, ct, mm] = qkv_w[j*256+mm, ct*128+p]
    wqk = np.ascontiguousarray(
        qkv_w[:2 * C].reshape(6, 256, NCT, 128).transpose(3, 0, 2, 1)
    ).astype(NP_BF16)
    # wv[p, ct, m] = qkv_w[2C+m, ct*128+p]
    wv = np.ascontiguousarray(
        qkv_w[2 * C:].reshape(C, NCT, 128).transpose(2, 1, 0)).astype(NP_BF16)
    # wp[p, ct, m] = proj_w[m, ct*128+p]
    wp = np.ascontiguousarray(
        proj_w.reshape(C, NCT, 128).transpose(2, 1, 0)).astype(NP_BF16)
    pbt = np.ascontiguousarray(proj_b.reshape(NCT, 128).T)  # [128, NCT] f32

    bpc = b // n_cores
    return [
        {"xt": xt[c * bpc:(c + 1) * bpc], "wqk": wqk, "wv": wv, "wp": wp,
         "pbt": pbt}
        for c in range(n_cores)
    ], bpc


def kernel(x1, x2, qkv_w, proj_w, proj_b):
    in_maps, bpc = make_in_maps(x1, x2, qkv_w, proj_w, proj_b)
    nc = _get_nc(bpc)
    res = run_bass_kernel_spmd(nc, in_maps, list(range(N_CORES)))
    yt = np.concatenate([res.results[c]["y"] for c in range(N_CORES)], axis=0)
    return np.ascontiguousarray(yt.transpose(0, 2, 1))

